# revision 16
# baseline (speedup 1.0000x reference)
"""Trainium2 Bass kernel for per-view cross-attention (v4).

Reference computation (per view v of 1024, S=64 samples, D=256):
  qp = q @ Wq.T + pe ; kp = k @ Wk.T + pe ; vp = v @ Wv.T + pe
  attn = softmax(qp @ kp.T / sqrt(D))
  x = gelu(attn @ vp @ Wo.T + bo) + q
Sharding: data-parallel over the 1024 views across 8 cores (128 views each).

Design notes:
- bf16 everywhere (DRAM I/O, SBUF, matmul operands; PSUM stays fp32).
  Halves HBM traffic and dodges the 4x cycles/row penalty on fp32 matmuls
  with small output free dims.
- Wo is folded into the v path on the host: vpo = v@(Wo@Wv).T + pe@Wo.T,
  so out = attn@vpo directly gives the pre-gelu activation (attn row-mixing
  commutes with Wo column-mixing). Kills the final projection entirely.
- scores are computed TRANSPOSED (operands swapped) and two views at a time
  with full 128-wide matmuls; only the two diagonal 64x64 blocks are valid,
  and the exp evacuation (one ACT op per half) reads just those.
- softmax reduction runs along partitions on PE: denominators via a [128,2]
  ones matmul (one per supertile), reciprocal on DVE, broadcast back across
  partitions with a [2,128] ones outer-product matmul (one per supertile).
- attn@vpo also runs two views per matmul using a block-diagonal normalized
  attnT tile ([128, 4, 128], off-diagonal zeroed once per tile by Pool).
- 4-stage software pipeline (A: load/proj/scoresT/exp; B: sums+recip;
  C: rec-broadcast+normalize; D: attn@vpo + gelu flush) so the in-order PE
  never waits on the ACT/DVE softmax chain. Stage A issues qp -> vpo -> kp
  so each PSUM ring slot has a full engine-burst of slack before reuse.
- engine balance per supertile (cost model): PE ~4.0us, DVE ~3.5, ACT ~3.6,
  Pool ~2.7. Pool (gpsimd) cannot access PSUM, so it gets SBUF-only work
  (residual adds, block-diag zeroing); pos-enc adds ride PSUM evacuations
  on DVE, except kp's, which is a third accumulation matmul on PE.
"""

import sys
import os

for p in ("/opt/trn_rl_repo",):
    if p not in sys.path and os.path.isdir(p):
        sys.path.insert(0, p)

import numpy as np
import ml_dtypes

BF16 = ml_dtypes.bfloat16

V, S, D = 1024, 64, 256
N_CORES = 8
VC = V // N_CORES          # views per core
ROWS = VC * S              # 8192 rows per core
R = 512                    # rows per supertile (8 views)
NST = ROWS // R            # supertiles per core
NV = R // S                # views per supertile
GELU_GROUP = 4             # supertiles per gelu flush (ACT table amortization)
LAG_B, LAG_C, LAG_D = 1, 2, 3
SCALE = 1.0 / np.sqrt(np.float32(D)).astype(np.float32)

_CACHE = {}


def _make_posenc(d_hid, n_samples):
    pos = np.arange(n_samples, dtype=np.float64)[:, None]
    j = np.arange(d_hid)[None, :]
    angle = pos / np.power(10000.0, 2.0 * (j // 2) / d_hid)
    table = np.where(j % 2 == 0, np.sin(angle), np.cos(angle))
    return table.astype(np.float32)  # [S, D]


def _build(rows=ROWS, gelu_copy=False, gelu_group=3, lags=(2, 3, 4),
           dep_hints=True, resid="split", qk_bufs=2, sm_bufs=3,
           psa_bufs=2, psb_bufs=3, store_q="sync", hoist_loads=True,
           ld_bufs=3):
    import concourse.bass as bass
    import concourse.mybir as mybir
    import concourse.tile as tile
    from concourse.tile import add_dep_helper
    from concourse import bacc
    from contextlib import ExitStack

    fp32 = mybir.dt.float32
    bf16 = mybir.dt.bfloat16
    AF = mybir.ActivationFunctionType
    ALU = mybir.AluOpType
    n_st = rows // R
    LAG_B, LAG_C, LAG_D = lags
    GG = gelu_group

    nc = bacc.Bacc(None, target_bir_lowering=False)

    qT_d = nc.dram_tensor("qT", [D, rows], bf16, kind="ExternalInput")
    kT_d = nc.dram_tensor("kT", [D, rows], bf16, kind="ExternalInput")
    vT_d = nc.dram_tensor("vT", [D, rows], bf16, kind="ExternalInput")
    wq_d = nc.dram_tensor("WqT", [D, D], bf16, kind="ExternalInput")
    wk_d = nc.dram_tensor("WkT", [D, D], bf16, kind="ExternalInput")
    wvo_d = nc.dram_tensor("WvoT", [D, D], bf16, kind="ExternalInput")
    bo_d = nc.dram_tensor("bo", [D], fp32, kind="ExternalInput")
    pet_d = nc.dram_tensor("peT_rep", [D, R], bf16, kind="ExternalInput")
    pen_d = nc.dram_tensor("pe_nat", [S, D], bf16, kind="ExternalInput")
    peo2_d = nc.dram_tensor("peo_nat2", [128, D], bf16, kind="ExternalInput")
    e8_d = nc.dram_tensor("E8", [S, R], bf16, kind="ExternalInput")
    ones2_d = nc.dram_tensor("ones2", [128, 2], bf16, kind="ExternalInput")
    ones2t_d = nc.dram_tensor("ones2T", [2, 128], bf16, kind="ExternalInput")
    out_d = nc.dram_tensor("outT", [D, rows], bf16, kind="ExternalOutput")

    def r3(ap):  # [D, X] dram -> [128, 2, X] partition view
        return ap.rearrange("(kc p) r -> p kc r", p=128)

    with tile.TileContext(nc) as tc, ExitStack() as ctx:
        ctx.enter_context(nc.allow_low_precision(
            reason="bf16 throughout is within the 2e-2 rel-err budget"))
        const = ctx.enter_context(tc.tile_pool(name="const", bufs=1))
        ld = ctx.enter_context(tc.tile_pool(name="ld", bufs=ld_bufs))
        proj = ctx.enter_context(tc.tile_pool(name="proj", bufs=2))
        sm = ctx.enter_context(tc.tile_pool(name="sm", bufs=sm_bufs))
        stg = ctx.enter_context(tc.tile_pool(name="stg", bufs=GG + 1))
        psA = ctx.enter_context(tc.tile_pool(name="psA", bufs=psa_bufs, space="PSUM"))
        psB = ctx.enter_context(tc.tile_pool(name="psB", bufs=psb_bufs, space="PSUM"))
        psS = ctx.enter_context(tc.tile_pool(name="psS", bufs=1, space="PSUM"))
        psR = ctx.enter_context(tc.tile_pool(name="psR", bufs=1, space="PSUM"))

        first = {}
        if hoist_loads:
            first["qt"] = ld.tile([128, 2, R], bf16, tag="qt",
                                  bufs=LAG_D + GG + 2, name="qt0")
            first["vt"] = ld.tile([128, 2, R], bf16, tag="vt", name="vt0")
            first["kt"] = ld.tile([128, 2, R], bf16, tag="kt", name="kt0")
            nc.sync.dma_start(first["qt"], r3(qT_d[:])[:, :, 0:R])
            nc.sync.dma_start(first["vt"], r3(vT_d[:])[:, :, 0:R])
            nc.sync.dma_start(first["kt"], r3(kT_d[:])[:, :, 0:R])
        wq = const.tile([128, 2, D], bf16)
        wk = const.tile([128, 2, D], bf16)
        wvo = const.tile([128, 2, D], bf16)
        nc.sync.dma_start(wq, r3(wq_d[:]))
        nc.sync.dma_start(wk, r3(wk_d[:]))
        nc.sync.dma_start(wvo, r3(wvo_d[:]))
        pet = const.tile([128, 2, R], bf16)
        nc.sync.dma_start(pet, r3(pet_d[:]))
        pen = const.tile([S, D], bf16)
        nc.sync.dma_start(pen, pen_d[:])
        peo2 = const.tile([128, D], bf16)
        nc.sync.dma_start(peo2, peo2_d[:])
        e8 = const.tile([S, R], bf16)
        nc.sync.dma_start(e8, e8_d[:])
        ones2 = const.tile([128, 2], bf16)
        nc.sync.dma_start(ones2, ones2_d[:])
        ones2t = const.tile([2, 128], bf16)
        nc.sync.dma_start(ones2t, ones2t_d[:])
        bo_sb = const.tile([128, 2], fp32)
        nc.sync.dma_start(bo_sb, bo_d.rearrange("(kc p) -> p kc", p=128))

        st_ctx = {}
        pending = []
        last_gelu = None
        last_exp = None
        for i in range(n_st + LAG_D):
            # -------- stage A: load, q/k/v projections, scoresT, exp --------
            if i < n_st:
                rs = slice(i * R, (i + 1) * R)
                if i == 0 and first:
                    qt, kt, vt = first["qt"], first["kt"], first["vt"]
                else:
                    qt = ld.tile([128, 2, R], bf16, tag="qt",
                                 bufs=LAG_D + GG + 2, name="qt")
                    kt = ld.tile([128, 2, R], bf16, tag="kt", name="kt")
                    vt = ld.tile([128, 2, R], bf16, tag="vt", name="vt")
                    nc.sync.dma_start(qt, r3(qT_d[:])[:, :, rs])
                    nc.sync.dma_start(vt, r3(vT_d[:])[:, :, rs])
                    nc.sync.dma_start(kt, r3(kT_d[:])[:, :, rs])

                # qp projection into transposed space qpT[dout, row];
                # pos-enc add rides the DVE evacuation.
                qpT = proj.tile([128, 2, R], bf16, tag="qpT", bufs=qk_bufs, name="qpT")
                kpT = proj.tile([128, 2, R], bf16, tag="kpT", bufs=qk_bufs, name="kpT")
                for mc in range(2):
                    ps = psA.tile([128, R], fp32, tag="psA", name="ps_qp")
                    for kc in range(2):
                        nc.tensor.matmul(
                            ps,
                            wq[:, kc, mc * 128:(mc + 1) * 128],
                            qt[:, kc, :],
                            start=(kc == 0),
                            stop=(kc == 1),
                        )
                    nc.vector.tensor_add(
                        out=qpT[:, mc, :], in0=ps, in1=pet[:, mc, :])

                # vpo = v@(Wo@Wv).T + pe@Wo.T, natural [row, dout] layout
                # (vt chunk as stationary); pos-enc add rides the evacuation.
                vpo = proj.tile([128, 4, D], bf16, tag="vpo",
                                bufs=LAG_D + 2, name="vpo")
                for g in range(4):
                    psv = psB.tile([128, 4, 128], fp32, tag="psB", name="ps_vpo")
                    pv = psv.rearrange("p a b -> p (a b)")[:, :D]
                    for kc in range(2):
                        nc.tensor.matmul(
                            pv,
                            vt[:, kc, g * 128:(g + 1) * 128],
                            wvo[:, kc, :],
                            start=(kc == 0),
                            stop=(kc == 1),
                        )
                    nc.vector.tensor_add(out=vpo[:, g, :], in0=pv, in1=peo2)

                # kp projection; pos-enc added on PE as a 3rd accumulation
                # matmul (pe_nat stationary, E8 one-hot rhs); ACT evacuates.
                for mc in range(2):
                    ps = psA.tile([128, R], fp32, tag="psA", name="ps_kp")
                    for kc in range(2):
                        nc.tensor.matmul(
                            ps,
                            wk[:, kc, mc * 128:(mc + 1) * 128],
                            kt[:, kc, :],
                            start=(kc == 0),
                            stop=False,
                        )
                    nc.tensor.matmul(
                        ps,
                        pen[:, mc * 128:(mc + 1) * 128],
                        e8,
                        start=False,
                        stop=True,
                    )
                    nc.scalar.copy(out=kpT[:, mc, :], in_=ps)

                # transposed scores, two views per matmul (full 128-wide):
                # scps[128(2 views k), g, 128(2 views q)]; only the diagonal
                # 64x64 blocks are meaningful.
                scps = psS.tile([128, 4, 128], fp32, tag="scps", name="scps")
                for g in range(4):
                    for dc in range(2):
                        nc.tensor.matmul(
                            scps[:, g, :],
                            kpT[:, dc, g * 128:(g + 1) * 128],
                            qpT[:, dc, g * 128:(g + 1) * 128],
                            start=(dc == 0),
                            stop=(dc == 1),
                        )

                # exp of the diagonal blocks only, into compact attnu
                # (no max-subtraction: |scores/16| < ~10)
                attnu = sm.tile([128, 4, S], bf16, tag="attnu",
                                bufs=LAG_C + 2, name="attnu")
                for h in range(2):
                    hs = slice(h * 64, (h + 1) * 64)
                    _e = nc.scalar.activation(
                        attnu[hs, :, :], scps[hs, :, h * 64:(h + 1) * 64],
                        AF.Exp, scale=float(SCALE))
                    if dep_hints and last_gelu is not None:
                        add_dep_helper(_e.ins, last_gelu, sync=False,
                                       reason="act-table grouping: exp after prior gelus")
                    last_exp = _e.ins
                st_ctx[i] = dict(qt=qt, vpo=vpo, attnu=attnu)

            # -------- stage B: softmax denominators (PE) + reciprocal ------
            jb = i - LAG_B
            if 0 <= jb < n_st:
                c = st_ctx[jb]
                sums = psR.tile([2, 4, S], fp32, tag="sums", name="sums",
                                padded_shape=[2, 4, 2 * S])
                nc.tensor.matmul(sums, ones2, c["attnu"], start=True, stop=True)
                rec = sm.tile([2, 4, S], bf16, tag="rec", name="rec")
                nc.vector.reciprocal(rec, sums)
                c["rec"] = rec

            # -------- stage C: broadcast reciprocal, normalize into
            # block-diagonal attnT2 (off-diagonal zeroed by Pool) -----------
            jc = i - LAG_C
            if 0 <= jc < n_st:
                c = st_ctx[jc]
                rrep = psR.tile([128, 4, S], fp32, tag="rrep", name="rrep",
                                padded_shape=[128, 4, 2 * S])
                nc.tensor.matmul(rrep, ones2t, c["rec"], start=True, stop=True)
                attnT2 = sm.tile([128, 4, 128], bf16, tag="attnT2", name="attnT2")
                nc.gpsimd.memset(attnT2, 0.0)
                for h in range(2):
                    hs = slice(h * 64, (h + 1) * 64)
                    nc.vector.tensor_tensor(
                        attnT2[hs, :, h * 64:(h + 1) * 64],
                        c["attnu"][hs, :, :], rrep[hs, :, :], ALU.mult)
                c["attnT2"] = attnT2

            # -------- stage D: attn@vpo -> pre-gelu, gelu flush ------------
            jd = i - LAG_D
            if 0 <= jd < n_st:
                c = st_ctx.pop(jd)
                pre = stg.tile([128, 2, R], bf16, tag="pre", name="pre")
                for cc in range(2):
                    pso = psB.tile([128, 4, 128], fp32, tag="psB", name="ps_av")
                    for g in range(4):
                        nc.tensor.matmul(
                            pso[:, g, :],
                            c["vpo"][:, g, cc * 128:(cc + 1) * 128],
                            c["attnT2"][:, g, :],
                            start=True,
                            stop=True,
                        )
                    # pso free layout [g, (two h, s)] == pre chunk layout
                    if cc == 0:
                        nc.scalar.copy(out=pre[:, cc, :],
                                       in_=pso.rearrange("p a b -> p (a b)"))
                    else:
                        nc.vector.tensor_copy(
                            pre[:, cc, :], pso.rearrange("p a b -> p (a b)"))
                pending.append((jd, pre, c["qt"]))

                if len(pending) == GG or jd == n_st - 1:
                    outs = []
                    for pst, ppre, pqt in pending:
                        outsb = proj.tile([128, 2, R], bf16, tag="outsb",
                                          bufs=GG + 1, name="outsb")
                        for mc in range(2):
                            if gelu_copy:
                                _g = nc.scalar.activation(
                                    out=outsb[:, mc, :], in_=ppre[:, mc, :],
                                    func=AF.Copy, bias=0.0, scale=1.0,
                                )
                            else:
                                _g = nc.scalar.activation(
                                    out=outsb[:, mc, :], in_=ppre[:, mc, :],
                                    func=AF.Gelu, bias=bo_sb[:, mc:mc + 1],
                                    scale=1.0,
                                )
                            if dep_hints and last_exp is not None:
                                add_dep_helper(_g.ins, last_exp, sync=False,
                                               reason="act-table grouping: gelu after group exps")
                            last_gelu = _g.ins
                            r_eng = (nc.gpsimd if resid == "pool" else
                                     nc.vector if resid == "dve" else
                                     (nc.vector if mc == 0 else nc.gpsimd))
                            r_eng.tensor_add(
                                out=outsb[:, mc, :], in0=outsb[:, mc, :],
                                in1=pqt[:, mc, :],
                            )
                        outs.append((pst, outsb))
                    st_eng = nc.scalar if store_q == "act" else nc.sync
                    for pst, outsb in outs:
                        st_eng.dma_start(
                            r3(out_d[:])[:, :, pst * R:(pst + 1) * R], outsb
                        )
                    pending = []

    nc.finalize()
    return nc


def _get_nc():
    if "nc" not in _CACHE:
        _CACHE["nc"] = _build()
    return _CACHE["nc"]


def _host_inputs(q, k, v, Wq, Wk, Wv, Wo, bo):
    pe = _make_posenc(D, S)                                   # [S, D] fp32
    Wo32 = np.asarray(Wo, np.float32)
    Wv32 = np.asarray(Wv, np.float32)
    Wvo = Wo32 @ Wv32                                         # fused v->out
    peo = pe @ Wo32.T                                         # pe through Wo
    peT_rep = np.ascontiguousarray(np.tile(pe.T, (1, NV))).astype(BF16)
    peo2 = np.ascontiguousarray(np.tile(peo, (2, 1))).astype(BF16)  # [128, D]
    e8 = np.ascontiguousarray(
        np.tile(np.eye(S, dtype=np.float32), (1, NV))).astype(BF16)  # [S, R]
    ones2 = np.zeros((128, 2), BF16)
    ones2[:64, 0] = 1
    ones2[64:, 1] = 1
    ones2t = np.ascontiguousarray(ones2.T)                    # [2, 128]
    consts = {
        "WqT": np.asarray(Wq, np.float32).T.astype(BF16),
        "WkT": np.asarray(Wk, np.float32).T.astype(BF16),
        "WvoT": Wvo.T.astype(BF16),
        "bo": np.ascontiguousarray(np.asarray(bo, np.float32)),
        "peT_rep": peT_rep,
        "pe_nat": pe.astype(BF16),
        "peo_nat2": peo2,
        "E8": e8,
        "ones2": ones2,
        "ones2T": ones2t,
    }
    consts = {k_: np.ascontiguousarray(v_) for k_, v_ in consts.items()}
    qb = np.asarray(q, np.float32).astype(BF16)
    kb = np.asarray(k, np.float32).astype(BF16)
    vb = np.asarray(v, np.float32).astype(BF16)
    in_maps = []
    for c in range(N_CORES):
        sl = slice(c * VC, (c + 1) * VC)
        m = dict(consts)
        m["qT"] = np.ascontiguousarray(qb[sl].reshape(ROWS, D).T)
        m["kT"] = np.ascontiguousarray(kb[sl].reshape(ROWS, D).T)
        m["vT"] = np.ascontiguousarray(vb[sl].reshape(ROWS, D).T)
        in_maps.append(m)
    return in_maps


def kernel(q, k, v, Wq, Wk, Wv, Wo, bo, _trace=False):
    from concourse.bass_utils import run_bass_kernel_spmd

    nc = _get_nc()
    in_maps = _host_inputs(q, k, v, Wq, Wk, Wv, Wo, bo)
    res = run_bass_kernel_spmd(nc, in_maps, list(range(N_CORES)), trace=_trace)
    outs = [
        np.asarray(res.results[c]["outT"], np.float32)
        .reshape(D, VC, S).transpose(1, 2, 0)
        for c in range(N_CORES)
    ]
    full = np.concatenate(outs, axis=0)
    if _trace:
        _CACHE["last_results"] = res
    return full
